# revision 2
# baseline (speedup 1.0000x reference)
"""Trainium2 Bass kernel for nn_AggregationAndDiscriminationLoss.

Data-parallel over batch: 2 images per core on 8 cores.

Device algorithm per image (N = 896*896 = 128 x 6272 pixels):
  - sim2 = sum_c sim_c^2          (ACT Square -> bf16, DVE adds)
  - bf16 one-hot mask tiles for labels 1..16 of T and K (DVE tensor_scalar
    is_equal -- fast multi-x perf mode)
  - "PE-diag fold": matmuls with stationary value-tile blocks
    [v(56) | v*(T==K)(56) | ones(1)] against moving mask tiles, PSUM
    accumulating over all pixel blocks.  The PSUM diagonal then holds
      row c      : sum v * mask_i        (A_i / Bk_i partials)
      row 56+c   : sum v*(T==K) * mask_i (Cc_i partials)
      row 112    : sum mask_i            (cT_i / cK_i partials)
  - PSUM -> DRAM; host sums diagonals in float64 and applies the tiny
    16-label loss formulas.

A BIR post-pass (_legalize_sync) splits multi-wait instructions into
NOP-carried single waits -- this walrus build rejects >1 sync wait per
instruction (Tile's drain/barrier emits up to 3).
"""

import numpy as np

import concourse.bass as bass
import concourse.tile as tile
from concourse import mybir
from concourse.bass_utils import run_bass_kernel_spmd

B, C, H, W = 16, 4, 896, 896
NCORES = 8
IMGS = B // NCORES          # images per core
P = 128
NFREE = (H * W) // P        # 6272
NCH = 8                     # chunks per image
CF = NFREE // NCH           # 784 columns per chunk
BC = 56                     # value-block width
NB = CF // BC               # 14 blocks per chunk
M = 2 * BC + 1              # 113 stationary columns: v | ones | vtk
MK = BC + 1                 # 57-column stationary for the K set: v | ones
K_MAX = 16
SIGMA_AGG = 0.5
SIGMA_DIS = 3.0


def _legalize_sync(nc):
    """Split >1-wait instructions: this walrus only encodes one sync wait."""
    for fn in nc.m.functions:
        for blk in fn.blocks:
            new = []
            for ins in blk.instructions:
                si = ins.sync_info
                if si is not None and len(si.on_wait) > 1:
                    waits = list(si.on_wait)
                    for k, w in enumerate(waits[:-1]):
                        nop = mybir.InstNoOp(name=f"{ins.name}-ws{k}", ins=[], outs=[])
                        nop.engine = ins.engine
                        nop.sync_info = mybir.SyncInfo(on_wait=[w], on_update=[])
                        new.append(nop)
                    ins.sync_info = mybir.SyncInfo(
                        on_wait=[waits[-1]], on_update=list(si.on_update)
                    )
                new.append(ins)
            blk.instructions = new


def _build_nc(reps=1, ablate=(), mask_sep=False):
    """ablate: subset of {"pe", "masks", "dve", "act"} to skip (bench only)."""
    nc = bass.Bass()
    dt = mybir.dt
    eq = mybir.AluOpType.is_equal

    sim = nc.dram_tensor("sim", [IMGS, C, P, NFREE], dt.float32, kind="ExternalInput")
    tl = nc.dram_tensor("tlab", [IMGS, P, NFREE], dt.int32, kind="ExternalInput")
    kl = nc.dram_tensor("klab", [IMGS, P, NFREE], dt.int32, kind="ExternalInput")
    acc_d = nc.dram_tensor(
        "acc", [IMGS, M, 2, 448], dt.float32, kind="ExternalOutput"
    )
    accK_d = nc.dram_tensor(
        "accK", [IMGS, M, 2, 448], dt.float32, kind="ExternalOutput"
    )

    with tile.TileContext(nc) as tc:
        with (
            tc.tile_pool(name="io", bufs=2) as io,
            tc.tile_pool(name="tmp", bufs=2) as tmp,
            tc.tile_pool(name="mks", bufs=2) as mks,
            tc.tile_pool(name="ps", bufs=2, space="PSUM") as ps,
        ):
            def _image(b):
                poT = ps.tile([M, 2, 512], dt.float32, tag="poT")
                poK = ps.tile([M, 2, 512], dt.float32, tag="poK")
                for ci in range(NCH):
                    cs = slice(ci * CF, (ci + 1) * CF)
                    chs = []
                    for c in range(C):
                        ch = io.tile([P, CF], dt.float32, tag=f"ch{c}")
                        nc.sync.dma_start(ch[:], sim[b, c, :, cs])
                        chs.append(ch)
                    ti = io.tile([P, CF], dt.int32, tag="ti")
                    nc.sync.dma_start(ti[:], tl[b, :, cs])
                    ki = io.tile([P, CF], dt.int32, tag="ki")
                    nc.sync.dma_start(ki[:], kl[b, :, cs])

                    sqs = []
                    for c in range(C):
                        sq = tmp.tile([P, CF], dt.bfloat16, tag=f"sq{c}")
                        if "act" not in ablate:
                            nc.scalar.activation(
                                sq[:], chs[c][:],
                                mybir.ActivationFunctionType.Square,
                            )
                        sqs.append(sq)
                    # labels as one [P, 2, CF] tile: T in slot 0, K in slot 1
                    tkb = tmp.tile([P, 2, CF], dt.bfloat16, tag="tkb")
                    if "act" not in ablate:
                        nc.scalar.copy(tkb[:, 0, :], ti[:])
                        nc.scalar.copy(tkb[:, 1, :], ki[:])
                    tb = tkb[:, 0, :]
                    kb = tkb[:, 1, :]

                    vt = tmp.tile([P, NB, 114], dt.bfloat16, tag="vt")
                    if "dve" in ablate and "pe" not in ablate:
                        nc.vector.memset(vt[:, 0, 0:1], 0.0)
                    if "dve" not in ablate:
                        s01 = tmp.tile([P, CF], dt.bfloat16, tag="s01")
                        nc.vector.tensor_add(s01[:], sqs[0][:], sqs[1][:])
                        s23 = tmp.tile([P, CF], dt.bfloat16, tag="s23")
                        nc.vector.tensor_add(s23[:], sqs[2][:], sqs[3][:])
                        nc.vector.memset(vt[:, :, BC : BC + 1], 1.0)
                        vs = vt[:, :, 0:BC]
                        nc.vector.tensor_add(
                            vs,
                            s01[:].rearrange("p (b c) -> p b c", c=BC),
                            s23[:].rearrange("p (b c) -> p b c", c=BC),
                        )
                        tkm = tmp.tile([P, CF], dt.bfloat16, tag="tkm")
                        nc.vector.tensor_tensor(tkm[:], tb, kb, eq)
                        nc.vector.tensor_mul(
                            vt[:, :, BC + 1 : 2 * BC + 1],
                            tkm[:].rearrange("p (b c) -> p b c", c=BC),
                            vs,
                        )

                    # one is_equal per label over the concatenated [T|K] tile
                    mk = mks.tile([P, 16, 2, CF], dt.bfloat16, tag="mk")
                    if "masks" in ablate and "pe" not in ablate:
                        nc.vector.memset(mk[:, 0, 0, 0:1], 0.0)
                    if "masks" not in ablate:
                        if mask_sep:
                            for i in range(K_MAX):
                                nc.vector.tensor_single_scalar(
                                    mk[:, i, 0, :], tb, float(i + 1), eq
                                )
                                nc.vector.tensor_single_scalar(
                                    mk[:, i, 1, :], kb, float(i + 1), eq
                                )
                        else:
                            for i in range(K_MAX):
                                nc.vector.tensor_single_scalar(
                                    mk[:, i, :, :], tkb[:], float(i + 1), eq
                                )

                    if "pe" not in ablate:
                        mTr = mk[:, :, 0, :].rearrange("p i (b c) -> p i b c", c=BC)
                        mKr = mk[:, :, 1, :].rearrange("p i (b c) -> p i b c", c=BC)
                        for blk in range(NB):
                            first = ci == 0 and blk == 0
                            last = ci == NCH - 1 and blk == NB - 1
                            lhs = vt[:, blk, 0:M]
                            for ih in range(2):
                                sl = slice(ih * 8, (ih + 1) * 8)
                                nc.tensor.matmul(
                                    poT[:, ih, 0:448].rearrange(
                                        "m (i c) -> m i c", c=BC
                                    ),
                                    lhs,
                                    mTr[:, sl, blk, :],
                                    start=first, stop=last,
                                    skip_group_check=True,
                                )
                                nc.tensor.matmul(
                                    poK[:, ih, 0:448].rearrange(
                                        "m (i c) -> m i c", c=BC
                                    ),
                                    lhs,
                                    mKr[:, sl, blk, :],
                                    start=first, stop=last,
                                    skip_group_check=True,
                                )
                soT = tmp.tile([M, 2, 448], dt.float32, tag="soT")
                soK = tmp.tile([M, 2, 448], dt.float32, tag="soK")
                if "pe" not in ablate:
                    nc.scalar.copy(soT[:], poT[:, :, 0:448])
                    nc.scalar.copy(soK[:], poK[:, :, 0:448])
                else:
                    nc.vector.memset(soT[:], 0.0)
                    nc.vector.memset(soK[:], 0.0)
                nc.sync.dma_start(acc_d[b], soT[:])
                nc.sync.dma_start(accK_d[b], soK[:])

            def _all_images():
                for b in range(IMGS):
                    _image(b)

            if reps == 1:
                _all_images()
            else:
                with tc.For_i(0, reps, 1):
                    _all_images()
    _legalize_sync(nc)
    return nc


_NC_CACHE = None


def _get_nc():
    global _NC_CACHE
    if _NC_CACHE is None:
        _NC_CACHE = _build_nc()
    return _NC_CACHE


def _decode(accT, accK):
    """accT: [IMGS, M, 2, 448]; accK: [IMGS, MK, 2, 448] ->
    per-image (A, Bk, Cc, cT, cK) each [16]."""
    out = []
    for b in range(IMGS):
        aT = accT[b].astype(np.float64).reshape(M, 16, BC)
        aK = accK[b].astype(np.float64).reshape(M, 16, BC)
        A = np.array([aT[c, :, c] for c in range(BC)]).sum(axis=0)
        Cc = np.array([aT[BC + 1 + c, :, c] for c in range(BC)]).sum(axis=0)
        cT = aT[BC, :, :].sum(axis=1)
        Bk = np.array([aK[c, :, c] for c in range(BC)]).sum(axis=0)
        cK = aK[BC, :, :].sum(axis=1)
        out.append((A, Bk, Cc, cT, cK))
    return out


def _finalize(per_image):
    """per_image: list of B tuples (A, Bk, Cc, cT, cK) -> float32 losses."""
    labels = np.arange(1, K_MAX + 1, dtype=np.float64)
    L_agg_tot = 0.0
    L_dis_tot = 0.0
    for A, Bk, Cc, cT, cK in per_image:
        nz = np.nonzero(cK > 0.5)[0]
        num_kernels = int(nz.max() + 1) if nz.size else 0
        valid = labels <= num_kernels

        denom = cK + 1.0
        x = A + Bk / (denom * denom) - 2.0 * Cc / denom
        pos = x > 0
        norm = np.where(pos, np.sqrt(np.where(pos, x, 1.0)), 0.0) - SIGMA_AGG
        agg_terms = np.log(norm * norm + 1.0) / (cT + 1.0)
        L_agg_tot += float(np.sum(np.where(valid, agg_terms, 0.0)))

        D = Bk / ((cK + 0.001) ** 2)
        S = D[:, None] + D[None, :]
        pair_mask = (labels[:, None] < labels[None, :]) & valid[None, :]
        pnorm = np.sqrt(np.where(pair_mask, S, 1.0))
        dnorm = SIGMA_DIS - pnorm
        dis_terms = np.log(dnorm * dnorm + 1.0)
        dis_sum = float(np.sum(np.where(pair_mask, dis_terms, 0.0)))
        if num_kernels > 1:
            nk = float(num_kernels)
            L_dis_tot += dis_sum / (nk * (nk - 1.0))
    return np.float32(L_agg_tot), np.float32(L_dis_tot)


def _in_maps(pred_similarities, text_mask_ndi_labels, kernel_mask_ndi_labels):
    sim = np.asarray(pred_similarities, dtype=np.float32).reshape(B, C, P, NFREE)
    T = np.asarray(text_mask_ndi_labels, dtype=np.int32).reshape(B, P, NFREE)
    K = np.asarray(kernel_mask_ndi_labels, dtype=np.int32).reshape(B, P, NFREE)

    in_maps = []
    for core in range(NCORES):
        s = slice(IMGS * core, IMGS * (core + 1))
        in_maps.append(
            {
                "sim": np.ascontiguousarray(sim[s]),
                "tlab": np.ascontiguousarray(T[s]),
                "klab": np.ascontiguousarray(K[s]),
            }
        )
    return in_maps


def _run(pred_similarities, text_mask_ndi_labels, kernel_mask_ndi_labels,
         trace=False):
    in_maps = _in_maps(
        pred_similarities, text_mask_ndi_labels, kernel_mask_ndi_labels
    )

    nc = _get_nc()
    res = run_bass_kernel_spmd(
        nc, in_maps, core_ids=list(range(NCORES)), trace=trace
    )

    per_image = []
    for core in range(NCORES):
        per_image.extend(_decode(res.results[core]["acc"], res.results[core]["accK"]))
    return _finalize(per_image), res


def kernel(pred_similarities, text_mask_ndi_labels, kernel_mask_ndi_labels):
    out, _ = _run(pred_similarities, text_mask_ndi_labels, kernel_mask_ndi_labels)
    return out



# revision 3
# speedup vs baseline: 1.3452x; 1.3452x over previous
"""Trainium2 Bass kernel for nn_AggregationAndDiscriminationLoss.

Data-parallel over batch: 2 images per core on 8 cores.

Device algorithm per image (N = 896*896 = 128 x 6272 pixels):
  - sim2 = sum_c sim_c^2          (ACT Square -> bf16, DVE adds)
  - bf16 one-hot mask tiles for labels 1..16 of T and K (DVE tensor_scalar
    is_equal -- fast multi-x perf mode)
  - "PE-diag fold": matmuls with stationary value-tile blocks
    [v(56) | v*(T==K)(56) | ones(1)] against moving mask tiles, PSUM
    accumulating over all pixel blocks.  The PSUM diagonal then holds
      row c      : sum v * mask_i        (A_i / Bk_i partials)
      row 56+c   : sum v*(T==K) * mask_i (Cc_i partials)
      row 112    : sum mask_i            (cT_i / cK_i partials)
  - PSUM -> DRAM; host sums diagonals in float64 and applies the tiny
    16-label loss formulas.

A BIR post-pass (_legalize_sync) splits multi-wait instructions into
NOP-carried single waits -- this walrus build rejects >1 sync wait per
instruction (Tile's drain/barrier emits up to 3).
"""

import numpy as np

import concourse.bass as bass
import concourse.tile as tile
from concourse import mybir
from concourse.bass_utils import run_bass_kernel_spmd

B, C, H, W = 16, 4, 896, 896
NCORES = 8
IMGS = B // NCORES          # images per core
P = 128
NFREE = (H * W) // P        # 6272
NCH = 8                     # chunks per image
CF = NFREE // NCH           # 784 columns per chunk
BC = 56                     # value-block width
NB = CF // BC               # 14 blocks per chunk
M = 2 * BC + 1              # 113 stationary columns: v | ones | vtk
MK = BC + 1                 # 57-column stationary for the K set: v | ones
K_MAX = 16
SIGMA_AGG = 0.5
SIGMA_DIS = 3.0


def _legalize_sync(nc):
    """Split >1-wait instructions: this walrus only encodes one sync wait."""
    for fn in nc.m.functions:
        for blk in fn.blocks:
            new = []
            for ins in blk.instructions:
                si = ins.sync_info
                if si is not None and len(si.on_wait) > 1:
                    waits = list(si.on_wait)
                    for k, w in enumerate(waits[:-1]):
                        nop = mybir.InstNoOp(name=f"{ins.name}-ws{k}", ins=[], outs=[])
                        nop.engine = ins.engine
                        nop.sync_info = mybir.SyncInfo(on_wait=[w], on_update=[])
                        new.append(nop)
                    ins.sync_info = mybir.SyncInfo(
                        on_wait=[waits[-1]], on_update=list(si.on_update)
                    )
                new.append(ins)
            blk.instructions = new


def _build_nc(reps=1, ablate=(), mask_sep=False):
    """ablate: subset of {"pe", "masks", "dve", "act"} to skip (bench only)."""
    nc = bass.Bass()
    dt = mybir.dt
    eq = mybir.AluOpType.is_equal

    sim = nc.dram_tensor("sim", [IMGS, C, P, NFREE], dt.float32, kind="ExternalInput")
    tl = nc.dram_tensor("tlab", [IMGS, P, NFREE], dt.int32, kind="ExternalInput")
    kl = nc.dram_tensor("klab", [IMGS, P, NFREE], dt.int32, kind="ExternalInput")
    acc_d = nc.dram_tensor(
        "acc", [IMGS, M, 2, 448], dt.float32, kind="ExternalOutput"
    )
    accK_d = nc.dram_tensor(
        "accK", [IMGS, M, 2, 448], dt.float32, kind="ExternalOutput"
    )

    with tile.TileContext(nc) as tc:
        with (
            tc.tile_pool(name="io", bufs=2) as io,
            tc.tile_pool(name="tmp", bufs=2) as tmp,
            tc.tile_pool(name="mks", bufs=2) as mks,
            tc.tile_pool(name="ps", bufs=2, space="PSUM") as ps,
        ):
            def _image(b):
                poT = ps.tile([M, 2, 512], dt.float32, tag="poT")
                poK = ps.tile([M, 2, 512], dt.float32, tag="poK")
                for ci in range(NCH):
                    cs = slice(ci * CF, (ci + 1) * CF)
                    chs = []
                    for c in range(C):
                        ch = io.tile([P, CF], dt.float32, tag=f"ch{c}")
                        nc.sync.dma_start(ch[:], sim[b, c, :, cs])
                        chs.append(ch)
                    ti = io.tile([P, CF], dt.int32, tag="ti")
                    nc.sync.dma_start(ti[:], tl[b, :, cs])
                    ki = io.tile([P, CF], dt.int32, tag="ki")
                    nc.sync.dma_start(ki[:], kl[b, :, cs])

                    sqs = []
                    for c in range(C):
                        sq = tmp.tile([P, CF], dt.bfloat16, tag=f"sq{c}")
                        if "act" not in ablate:
                            nc.scalar.activation(
                                sq[:], chs[c][:],
                                mybir.ActivationFunctionType.Square,
                            )
                        elif "dve" not in ablate:
                            nc.vector.memset(sq[:, 0:1], 0.0)
                        sqs.append(sq)
                    # labels as one [P, 2, CF] tile: T in slot 0, K in slot 1
                    tkb = tmp.tile([P, 2, CF], dt.bfloat16, tag="tkb")
                    if "act" not in ablate:
                        nc.scalar.copy(tkb[:, 0, :], ti[:])
                        nc.scalar.copy(tkb[:, 1, :], ki[:])
                    elif "dve" not in ablate or "masks" not in ablate:
                        nc.vector.memset(tkb[:, 0, 0:1], 0.0)
                    tb = tkb[:, 0, :]
                    kb = tkb[:, 1, :]

                    vt = tmp.tile([P, NB, 114], dt.bfloat16, tag="vt")
                    if "dve" in ablate and "pe" not in ablate:
                        nc.vector.memset(vt[:, 0, 0:1], 0.0)
                    if "dve" not in ablate:
                        s01 = tmp.tile([P, CF], dt.bfloat16, tag="s01")
                        nc.vector.tensor_add(s01[:], sqs[0][:], sqs[1][:])
                        s23 = tmp.tile([P, CF], dt.bfloat16, tag="s23")
                        nc.vector.tensor_add(s23[:], sqs[2][:], sqs[3][:])
                        nc.vector.memset(vt[:, :, BC : BC + 1], 1.0)
                        vs = vt[:, :, 0:BC]
                        nc.vector.tensor_add(
                            vs,
                            s01[:].rearrange("p (b c) -> p b c", c=BC),
                            s23[:].rearrange("p (b c) -> p b c", c=BC),
                        )
                        tkm = tmp.tile([P, CF], dt.bfloat16, tag="tkm")
                        nc.vector.tensor_tensor(tkm[:], tb, kb, eq)
                        nc.vector.tensor_mul(
                            vt[:, :, BC + 1 : 2 * BC + 1],
                            tkm[:].rearrange("p (b c) -> p b c", c=BC),
                            vs,
                        )

                    # one is_equal per label over the concatenated [T|K] tile
                    mk = mks.tile([P, 16, 2, CF], dt.bfloat16, tag="mk")
                    if "masks" in ablate and "pe" not in ablate:
                        nc.vector.memset(mk[:, 0, 0, 0:1], 0.0)
                    if "masks" not in ablate:
                        if mask_sep:
                            for i in range(K_MAX):
                                nc.vector.tensor_single_scalar(
                                    mk[:, i, 0, :], tb, float(i + 1), eq
                                )
                                nc.vector.tensor_single_scalar(
                                    mk[:, i, 1, :], kb, float(i + 1), eq
                                )
                        else:
                            for i in range(K_MAX):
                                nc.vector.tensor_single_scalar(
                                    mk[:, i, :, :], tkb[:], float(i + 1), eq
                                )

                    if "pe" not in ablate:
                        mTr = mk[:, :, 0, :].rearrange("p i (b c) -> p i b c", c=BC)
                        mKr = mk[:, :, 1, :].rearrange("p i (b c) -> p i b c", c=BC)
                        for blk in range(NB):
                            first = ci == 0 and blk == 0
                            last = ci == NCH - 1 and blk == NB - 1
                            lhs = vt[:, blk, 0:M]
                            for ih in range(2):
                                sl = slice(ih * 8, (ih + 1) * 8)
                                nc.tensor.matmul(
                                    poT[:, ih, 0:448].rearrange(
                                        "m (i c) -> m i c", c=BC
                                    ),
                                    lhs,
                                    mTr[:, sl, blk, :],
                                    start=first, stop=last,
                                    skip_group_check=True,
                                )
                                nc.tensor.matmul(
                                    poK[:, ih, 0:448].rearrange(
                                        "m (i c) -> m i c", c=BC
                                    ),
                                    lhs,
                                    mKr[:, sl, blk, :],
                                    start=first, stop=last,
                                    skip_group_check=True,
                                )
                soT = tmp.tile([M, 2, 448], dt.float32, tag="soT")
                soK = tmp.tile([M, 2, 448], dt.float32, tag="soK")
                if "pe" not in ablate:
                    nc.scalar.copy(soT[:], poT[:, :, 0:448])
                    nc.scalar.copy(soK[:], poK[:, :, 0:448])
                else:
                    nc.vector.memset(soT[:], 0.0)
                    nc.vector.memset(soK[:], 0.0)
                nc.sync.dma_start(acc_d[b], soT[:])
                nc.sync.dma_start(accK_d[b], soK[:])

            def _all_images():
                for b in range(IMGS):
                    _image(b)

            if reps == 1:
                _all_images()
            else:
                with tc.For_i(0, reps, 1):
                    _all_images()
    _legalize_sync(nc)
    return nc


_NC_CACHE = None


def _get_nc():
    global _NC_CACHE
    if _NC_CACHE is None:
        _NC_CACHE = _build_nc()
    return _NC_CACHE


def _decode(accT, accK):
    """accT: [IMGS, M, 2, 448]; accK: [IMGS, MK, 2, 448] ->
    per-image (A, Bk, Cc, cT, cK) each [16]."""
    out = []
    for b in range(IMGS):
        aT = accT[b].astype(np.float64).reshape(M, 16, BC)
        aK = accK[b].astype(np.float64).reshape(M, 16, BC)
        A = np.array([aT[c, :, c] for c in range(BC)]).sum(axis=0)
        Cc = np.array([aT[BC + 1 + c, :, c] for c in range(BC)]).sum(axis=0)
        cT = aT[BC, :, :].sum(axis=1)
        Bk = np.array([aK[c, :, c] for c in range(BC)]).sum(axis=0)
        cK = aK[BC, :, :].sum(axis=1)
        out.append((A, Bk, Cc, cT, cK))
    return out


def _finalize(per_image):
    """per_image: list of B tuples (A, Bk, Cc, cT, cK) -> float32 losses."""
    labels = np.arange(1, K_MAX + 1, dtype=np.float64)
    L_agg_tot = 0.0
    L_dis_tot = 0.0
    for A, Bk, Cc, cT, cK in per_image:
        nz = np.nonzero(cK > 0.5)[0]
        num_kernels = int(nz.max() + 1) if nz.size else 0
        valid = labels <= num_kernels

        denom = cK + 1.0
        x = A + Bk / (denom * denom) - 2.0 * Cc / denom
        pos = x > 0
        norm = np.where(pos, np.sqrt(np.where(pos, x, 1.0)), 0.0) - SIGMA_AGG
        agg_terms = np.log(norm * norm + 1.0) / (cT + 1.0)
        L_agg_tot += float(np.sum(np.where(valid, agg_terms, 0.0)))

        D = Bk / ((cK + 0.001) ** 2)
        S = D[:, None] + D[None, :]
        pair_mask = (labels[:, None] < labels[None, :]) & valid[None, :]
        pnorm = np.sqrt(np.where(pair_mask, S, 1.0))
        dnorm = SIGMA_DIS - pnorm
        dis_terms = np.log(dnorm * dnorm + 1.0)
        dis_sum = float(np.sum(np.where(pair_mask, dis_terms, 0.0)))
        if num_kernels > 1:
            nk = float(num_kernels)
            L_dis_tot += dis_sum / (nk * (nk - 1.0))
    return np.float32(L_agg_tot), np.float32(L_dis_tot)


def _in_maps(pred_similarities, text_mask_ndi_labels, kernel_mask_ndi_labels):
    sim = np.asarray(pred_similarities, dtype=np.float32).reshape(B, C, P, NFREE)
    T = np.asarray(text_mask_ndi_labels, dtype=np.int32).reshape(B, P, NFREE)
    K = np.asarray(kernel_mask_ndi_labels, dtype=np.int32).reshape(B, P, NFREE)

    in_maps = []
    for core in range(NCORES):
        s = slice(IMGS * core, IMGS * (core + 1))
        in_maps.append(
            {
                "sim": np.ascontiguousarray(sim[s]),
                "tlab": np.ascontiguousarray(T[s]),
                "klab": np.ascontiguousarray(K[s]),
            }
        )
    return in_maps


def _run(pred_similarities, text_mask_ndi_labels, kernel_mask_ndi_labels,
         trace=False):
    in_maps = _in_maps(
        pred_similarities, text_mask_ndi_labels, kernel_mask_ndi_labels
    )

    nc = _get_nc()
    res = run_bass_kernel_spmd(
        nc, in_maps, core_ids=list(range(NCORES)), trace=trace
    )

    per_image = []
    for core in range(NCORES):
        per_image.extend(_decode(res.results[core]["acc"], res.results[core]["accK"]))
    return _finalize(per_image), res


def kernel(pred_similarities, text_mask_ndi_labels, kernel_mask_ndi_labels):
    out, _ = _run(pred_similarities, text_mask_ndi_labels, kernel_mask_ndi_labels)
    return out



# revision 5
# speedup vs baseline: 1.4359x; 1.0674x over previous
"""Trainium2 Bass kernel v2 for nn_AggregationAndDiscriminationLoss.

Data-parallel over batch: 2 images per core on 8 cores.

Per image (N = 896*896 pixels as [128, 6272]), per chunk of CF=784 cols:
  - ACT: sq_c = sim_c^2 (4 ops), tkb = bf16(labels) (1 op), plus the
    label-16 one-hot via relu(1-(x-16)^2) (2 ops) -- exact for int labels.
  - DVE: s01/s23 adds, vs -> vt[:, :, 0:28], tkm = [T==K],
    vtk = vs*tkm -> vt[:, :, 28:56], ones col, 15 is_equal mask ops
    (4x perf mode, 440 cyc each).
  - PE:  2-group column tiling.  Stationary vt[blk] = [v(28)|vtk(28)|1]
    (57 cols); group g = blk%2 targets PE columns/PSUM partitions 64g..
    64g+56.  One matmul per (blk, map): moving = mk[:, :, m, blk cols]
    (16 labels x 28 cols = 448 cols), accumulated into po[64g:64g+57, m]
    over the whole image.  Concurrent col-groups -> ~2x PE throughput.
  - per image: ACT evac po[0:121] -> so, DMA out.

Host decodes diagonals (A, Bk, Cc, cT, cK per label) in f64 and applies
the 16-label loss formulas.
"""

import numpy as np

import concourse.bass as bass
import concourse.tile as tile
from concourse import mybir
from concourse.bass_utils import run_bass_kernel_spmd

B, C, H, W = 16, 4, 896, 896
NCORES = 8
IMGS = B // NCORES
P = 128
NFREE = (H * W) // P        # 6272
NCH = 8                     # chunks per image
CF = NFREE // NCH           # 784
BC = 28                     # value-block width
NB = CF // BC               # 28 blocks per chunk
M = 2 * BC + 1              # 57 stationary cols: v | vtk | ones
MP = 58                     # padded block stride (4B-aligned strides)
K_MAX = 16
ACT_LABELS = (16,)          # labels whose masks are built on ScalarE
SIGMA_AGG = 0.5
SIGMA_DIS = 3.0


def _legalize_sync(nc):
    """Split >1-wait instructions: this walrus only encodes one sync wait."""
    for fn in nc.m.functions:
        for blk in fn.blocks:
            new = []
            for ins in blk.instructions:
                si = ins.sync_info
                if si is not None and len(si.on_wait) > 1:
                    waits = list(si.on_wait)
                    for k, w in enumerate(waits[:-1]):
                        nop = mybir.InstNoOp(name=f"{ins.name}-ws{k}", ins=[], outs=[])
                        nop.engine = ins.engine
                        nop.sync_info = mybir.SyncInfo(on_wait=[w], on_update=[])
                        new.append(nop)
                    ins.sync_info = mybir.SyncInfo(
                        on_wait=[waits[-1]], on_update=list(si.on_update)
                    )
                new.append(ins)
            blk.instructions = new


def _build_nc(reps=1, ablate=(), ch_bufs=2, mm_halves=2, act_labels=None):
    ACT_L = ACT_LABELS if act_labels is None else act_labels
    nc = bass.Bass()
    dt = mybir.dt
    eq = mybir.AluOpType.is_equal
    AF = mybir.ActivationFunctionType

    sim = nc.dram_tensor("sim", [IMGS, C, P, NFREE], dt.float32, kind="ExternalInput")
    tl = nc.dram_tensor("tlab", [IMGS, P, NFREE], dt.int32, kind="ExternalInput")
    kl = nc.dram_tensor("klab", [IMGS, P, NFREE], dt.int32, kind="ExternalInput")
    acc_d = nc.dram_tensor(
        "acc", [IMGS, 121, 2, 448], dt.float32, kind="ExternalOutput"
    )

    with tile.TileContext(nc) as tc:
        with (
            tc.tile_pool(name="cst", bufs=1) as cst,
            tc.tile_pool(name="io", bufs=2) as io,
            tc.tile_pool(name="tmp", bufs=2) as tmp,
            tc.tile_pool(name="mks", bufs=2) as mks,
            tc.tile_pool(name="ps", bufs=2, space="PSUM") as ps,
        ):
            bias_t = cst.tile([P, len(ACT_L) + 1], mybir.dt.float32)
            for j, l in enumerate(ACT_L):
                nc.vector.memset(bias_t[:, j:j + 1], float(-l))
            nc.vector.memset(bias_t[:, len(ACT_L):], 1.0)
            # static double-buffered stationary tiles; ones col written once
            vts = []
            for v in range(2):
                vt_s = cst.tile([P, NB, MP], mybir.dt.bfloat16, name=f"vt{v}")
                nc.vector.memset(vt_s[:, :, 2 * BC:M], 1.0)
                vts.append(vt_s)

            def _image(b):
                po = ps.tile([128, 2, 512], dt.float32, tag="po")
                for ci in range(NCH):
                    cs = slice(ci * CF, (ci + 1) * CF)
                    lab = io.tile([P, 2, CF], dt.int32, tag="lab")
                    nc.sync.dma_start(lab[:, 0, :], tl[b, :, cs])
                    nc.sync.dma_start(lab[:, 1, :], kl[b, :, cs])
                    chs = []
                    for c in range(C):
                        ch = io.tile([P, CF], dt.float32, tag=f"ch{c}", bufs=ch_bufs)
                        nc.sync.dma_start(ch[:], sim[b, c, :, cs])
                        chs.append(ch)

                    # --- ScalarE: tkb first (feeds the DVE mask burst) ---
                    tkb = tmp.tile([P, 2, CF], dt.bfloat16, tag="tkb")
                    if "act" not in ablate:
                        nc.scalar.copy(tkb[:], lab[:])
                    elif "dve" not in ablate or "masks" not in ablate:
                        nc.vector.memset(tkb[:, 0, 0:1], 0.0)

                    mk = mks.tile([P, 16, 2, CF], dt.bfloat16, tag="mk")
                    if "pe" not in ablate and (
                        "masks" in ablate or "act" in ablate
                    ):
                        nc.vector.memset(mk[:, 0, 0, 0:1], 0.0)
                    ats = []
                    if "masks" not in ablate and "act" not in ablate:
                        # exact one-hot for integer labels: relu(1-2*(x-i)^2)
                        for j, l in enumerate(ACT_L):
                            at = tmp.tile([P, 2, CF], dt.bfloat16, tag=f"at{j}")
                            nc.scalar.activation(
                                at[:], tkb[:], AF.Square,
                                bias=bias_t[:, j:j + 1],
                            )
                            ats.append(at)
                    sqs = []
                    for c in range(C):
                        sq = tmp.tile([P, CF], dt.bfloat16, tag=f"sq{c}")
                        if "act" not in ablate:
                            nc.scalar.activation(sq[:], chs[c][:], AF.Square)
                        elif "dve" not in ablate:
                            nc.vector.memset(sq[:, 0:1], 0.0)
                        sqs.append(sq)
                    if "masks" not in ablate and "act" not in ablate:
                        for j, l in enumerate(ACT_L):
                            # scale=-2: tolerates ACT Square's 1-ULP error
                            # at at==1 (neighbor labels) while staying exact
                            # at at==0 (the matching label).
                            nc.scalar.activation(
                                mk[:, l - 1, :, :], ats[j][:], AF.Relu,
                                bias=bias_t[:, len(ACT_L):], scale=-2.0,
                            )

                    # --- VectorE: mask burst first, value assembly after ---
                    vt = vts[ci % 2]
                    if "dve" not in ablate:
                        tkm = tmp.tile([P, CF], dt.bfloat16, tag="tkm")
                        nc.vector.tensor_tensor(
                            tkm[:], tkb[:, 0, :], tkb[:, 1, :], eq
                        )
                    if "masks" not in ablate:
                        for l in range(1, K_MAX + 1):
                            if l in ACT_L and "act" not in ablate:
                                continue
                            nc.vector.tensor_single_scalar(
                                mk[:, l - 1, :, :], tkb[:], float(l), eq
                            )
                    if "dve" not in ablate:
                        s01 = tmp.tile([P, CF], dt.bfloat16, tag="s01")
                        nc.vector.tensor_add(s01[:], sqs[0][:], sqs[1][:])
                        s23 = tmp.tile([P, CF], dt.bfloat16, tag="s23")
                        nc.vector.tensor_add(s23[:], sqs[2][:], sqs[3][:])
                        vs = vt[:, :, 0:BC]
                        nc.vector.tensor_add(
                            vs,
                            s01[:].rearrange("p (b c) -> p b c", c=BC),
                            s23[:].rearrange("p (b c) -> p b c", c=BC),
                        )
                        nc.vector.tensor_mul(
                            vt[:, :, BC:2 * BC],
                            vs,
                            tkm[:].rearrange("p (b c) -> p b c", c=BC),
                        )

                    # --- TensorE: 2-group column tiling ---
                    if "pe" not in ablate:
                        for blk in range(NB):
                            g = blk % 2
                            rows = slice(64 * g, 64 * g + M)
                            first = ci == 0 and blk < 2
                            last = ci == NCH - 1 and blk >= NB - 2
                            lhs = vt[:, blk, 0:M]
                            bs = slice(blk * BC, (blk + 1) * BC)
                            nh = mm_halves
                            hl = 16 // nh
                            for m in range(2):
                                for h in range(nh):
                                    nc.tensor.matmul(
                                        po[rows, m, h * hl * BC:(h + 1) * hl * BC]
                                        .rearrange("q (i c) -> q i c", c=BC),
                                        lhs,
                                        mk[:, h * hl:(h + 1) * hl, m, bs],
                                        start=first, stop=last,
                                        skip_group_check=True,
                                    )
                so = tmp.tile([121, 2, 448], dt.float32, tag="so")
                if "pe" not in ablate:
                    nc.scalar.copy(so[:], po[0:121, :, 0:448])
                else:
                    nc.vector.memset(so[:, :, 0:1], 0.0)
                nc.sync.dma_start(acc_d[b], so[:])

            def _all_images():
                for b in range(IMGS):
                    _image(b)

            if reps == 1:
                _all_images()
            else:
                with tc.For_i(0, reps, 1):
                    _all_images()
    _legalize_sync(nc)
    return nc


_NC_CACHE = None


def _get_nc():
    global _NC_CACHE
    if _NC_CACHE is None:
        _NC_CACHE = _build_nc()
    return _NC_CACHE


def _decode(acc):
    """acc: [IMGS, 121, 2, 448] -> per-image (A, Bk, Cc, cT, cK) each [16]."""
    out = []
    for b in range(IMGS):
        a = acc[b].astype(np.float64).reshape(121, 2, 16, BC)
        A = np.zeros(16)
        Bk = np.zeros(16)
        Cc = np.zeros(16)
        cT = np.zeros(16)
        cK = np.zeros(16)
        for g in (0, 1):
            r0 = 64 * g
            for c in range(BC):
                A += a[r0 + c, 0, :, c]
                Bk += a[r0 + c, 1, :, c]
                Cc += a[r0 + BC + c, 1, :, c]
            cT += a[r0 + 2 * BC, 0, :, :].sum(axis=1)
            cK += a[r0 + 2 * BC, 1, :, :].sum(axis=1)
        out.append((A, Bk, Cc, cT, cK))
    return out


def _finalize(per_image):
    labels = np.arange(1, K_MAX + 1, dtype=np.float64)
    L_agg_tot = 0.0
    L_dis_tot = 0.0
    for A, Bk, Cc, cT, cK in per_image:
        nz = np.nonzero(cK > 0.5)[0]
        num_kernels = int(nz.max() + 1) if nz.size else 0
        valid = labels <= num_kernels

        denom = cK + 1.0
        x = A + Bk / (denom * denom) - 2.0 * Cc / denom
        pos = x > 0
        norm = np.where(pos, np.sqrt(np.where(pos, x, 1.0)), 0.0) - SIGMA_AGG
        agg_terms = np.log(norm * norm + 1.0) / (cT + 1.0)
        L_agg_tot += float(np.sum(np.where(valid, agg_terms, 0.0)))

        D = Bk / ((cK + 0.001) ** 2)
        S = D[:, None] + D[None, :]
        pair_mask = (labels[:, None] < labels[None, :]) & valid[None, :]
        pnorm = np.sqrt(np.where(pair_mask, S, 1.0))
        dnorm = SIGMA_DIS - pnorm
        dis_terms = np.log(dnorm * dnorm + 1.0)
        dis_sum = float(np.sum(np.where(pair_mask, dis_terms, 0.0)))
        if num_kernels > 1:
            nk = float(num_kernels)
            L_dis_tot += dis_sum / (nk * (nk - 1.0))
    return np.float32(L_agg_tot), np.float32(L_dis_tot)


def _in_maps(pred_similarities, text_mask_ndi_labels, kernel_mask_ndi_labels):
    sim = np.asarray(pred_similarities, dtype=np.float32).reshape(B, C, P, NFREE)
    T = np.asarray(text_mask_ndi_labels, dtype=np.int32).reshape(B, P, NFREE)
    K = np.asarray(kernel_mask_ndi_labels, dtype=np.int32).reshape(B, P, NFREE)

    in_maps = []
    for core in range(NCORES):
        s = slice(IMGS * core, IMGS * (core + 1))
        in_maps.append(
            {
                "sim": np.ascontiguousarray(sim[s]),
                "tlab": np.ascontiguousarray(T[s]),
                "klab": np.ascontiguousarray(K[s]),
            }
        )
    return in_maps


def _run(pred_similarities, text_mask_ndi_labels, kernel_mask_ndi_labels,
         trace=False):
    in_maps = _in_maps(
        pred_similarities, text_mask_ndi_labels, kernel_mask_ndi_labels
    )
    nc = _get_nc()
    res = run_bass_kernel_spmd(
        nc, in_maps, core_ids=list(range(NCORES)), trace=trace
    )
    per_image = []
    for core in range(NCORES):
        per_image.extend(_decode(res.results[core]["acc"]))
    return _finalize(per_image), res


def kernel(pred_similarities, text_mask_ndi_labels, kernel_mask_ndi_labels):
    out, _ = _run(pred_similarities, text_mask_ndi_labels, kernel_mask_ndi_labels)
    return out
